# revision 22
# baseline (speedup 1.0000x reference)
"""GCN message-passing kernel for 8 Trainium2 NeuronCores.

out = log_softmax(mean_agg(norm * (x@W)[src] -> dst) + b)

Strategy (graph/data parallel per the sharding hint):
  - Shard dst nodes (and their incoming edges) across 8 cores; within a
    core, dst node i maps to block i//128, lane i%128 (98 blocks).
  - Phase A: each core computes xw = x_shard @ W (PE, bf16), scales by
    deg^-1/2 into y (bf16), stores its y shard to a 256-byte-strided
    DRAM table (row = 128B of y + 128B pad, to satisfy dma_gather's
    256-byte element-size requirement).
  - Phase B: two AllGathers (half-shards) replicate the y table so every
    core has all 100352 rows; the second half overlaps with Phase C.
  - Phase C: edges sorted by (source-range group, dst block) on host.
    A few large dma_gather calls (16 idxs/descriptor, ~1us fixed cost
    each) pull y[src] rows into SBUF; DVE builds one-hot(dst_lane)
    tiles; PE matmuls aggregate each (group, block) segment in PSUM,
    drained into an SBUF f32 accumulator; per-block epilogue applies
    deg^-3/2 scaling, self-loop, bias, and log_softmax.

Math identity used (self-loops make deg >= 1 and cnt == deg):
  out[d] = deg[d]^-3/2 * (sum_{e: dst=d} y[src_e] + y[d]) + b
  with y[n] = xw[n] * deg[n]^-1/2, followed by row log_softmax.
"""

from contextlib import ExitStack

import numpy as np
import ml_dtypes

import concourse.bacc as bacc
import concourse.bass as bass
import concourse.mybir as mybir
import concourse.tile as tile
from concourse import bass_utils

# Problem sizes (hardcoded per the harness contract).
N = 100000
F = 256
C = 64
E = 3200000
N_CORES = 8
NSH = N // N_CORES          # 12500 dst nodes per core
PB = (NSH + 127) // 128     # 98 blocks of 128 dst nodes
NP = PB * 128               # padded shard rows (12544)
HALF = NP // 2              # 6272 rows: half-shard for the split AllGather
GROUP = NP * N_CORES // 4   # 25088 rows per gather group (int16-indexable)
TC = 80                     # gather-chunk size in 128-edge tiles
EG = 7                      # output blocks per store DMA

f32 = mybir.dt.float32
bf16 = mybir.dt.bfloat16
i32 = mybir.dt.int32
i16 = mybir.dt.int16
AF = mybir.ActivationFunctionType


NHH = NSH // 2                  # 6250 nodes per fixed half
HB = PB // 2                    # 49 blocks per half (49*128 = 6272 = HALF)


def _balance(loads, nblk, cap=1016):
    """Greedy node->block assignment keeping per-group in-edge loads under
    cap (so SPMD segments stay at 8 tiles); the last block of the range is
    the designated overflow dump, aligned across cores."""
    order = np.argsort(-loads.sum(1), kind="stable")
    load = np.zeros((nblk, 4), dtype=np.int64)
    fill = np.zeros(nblk, dtype=np.int64)
    slot = np.zeros(len(loads), dtype=np.int64)
    for n in order:
        ln = loads[n]
        newl = load + ln
        over = (newl > cap).sum(axis=1).astype(np.float64)
        over[nblk - 1] = 0.0
        score = over * 1e9 + np.max(newl, axis=1)
        score[fill >= 128] = np.inf
        b = int(np.argmin(score))
        slot[n] = b * 128 + fill[b]
        fill[b] += 1
        load[b] += ln
    return slot


def build_layout(edge_index):
    """Shared host-side layout: edge -> (core, slot) with the tile stream
    sorted by (group, block), segments padded so all cores share one SPMD
    tile map. Returns layout dict + per-core slot fill data."""
    src = np.asarray(edge_index[0], dtype=np.int64)
    dst = np.asarray(edge_index[1], dtype=np.int64)

    core = dst // NSH
    dloc = dst % NSH
    scre = src // NSH
    sloc = src % NSH
    # Fixed half pre-split (local idx // 6250) makes a source's gather
    # group = 2*half + (core >= 4) independent of the within-half block
    # assignment, so block balancing can't shift groups.
    half_s = sloc // NHH
    grp = half_s * 2 + (scre // 4)
    # per-node in-edge loads by source group -> balanced blocks per half
    nl = np.zeros((N, 4), dtype=np.int64)
    np.add.at(nl, (dst, grp), 1)
    slot_all = np.zeros((N_CORES, NSH), dtype=np.int64)
    for c in range(N_CORES):
        cl = nl[c * NSH:(c + 1) * NSH]
        slot_all[c, :NHH] = _balance(cl[:NHH], HB)
        slot_all[c, NHH:] = HALF + _balance(cl[NHH:], HB)

    d_slot = slot_all[core, dloc]
    blk = d_slot // 128
    lane = d_slot % 128
    sub = blk % 2                  # which block of the pair
    pr = blk // 2                  # block pair 0..48
    s_slot = slot_all[scre, sloc]
    p_tab = half_s * (4 * NP) + scre * HALF + (s_slot - half_s * HALF)
    assert (p_tab // GROUP == grp).all()
    idxg = p_tab % GROUP           # int16-safe (< 25088)

    # per (core, group, block-pair) edge counts -> shared segment tiles.
    # Pair segments halve the segment count so the ceil + max-over-cores
    # padding amortizes over ~2048-edge segments; each tile aggregates to
    # both blocks of the pair via a 256-wide one-hot (2 matmuls/tile).
    npair = PB // 2
    nseg_tot = 4 * npair
    sid_e = grp * npair + pr
    csid = core * nseg_tot + sid_e
    cnt = np.bincount(csid, minlength=N_CORES * nseg_tot).reshape(
        N_CORES, nseg_tot)
    maxc = cnt.max(axis=0)         # shared (SPMD) real idx count per segment
    ts = (np.ceil(maxc / 128).astype(np.int64))  # tiles per segment
    seg_t0 = np.zeros(nseg_tot, dtype=np.int64)
    np.cumsum(ts[:-1], out=seg_t0[1:])
    t_total = int(ts.sum())

    # slot of each edge: segment base + rank within (core, segment)
    order = np.lexsort((sid_e, core))
    inv_start = np.zeros(N_CORES * nseg_tot, dtype=np.int64)
    np.cumsum(cnt.reshape(-1)[:-1], out=inv_start[1:])
    rank = np.arange(len(dst), dtype=np.int64) - inv_start[csid[order]]
    slot = seg_t0[sid_e[order]] * 128 + rank

    # idx fill: real edges, 0-padding (gathered, masked by zero one-hot);
    # lane value encodes block parity: lane + 128*sub in [0, 256)
    idx16 = np.zeros((N_CORES, t_total * 128), dtype=np.int16)
    lanef = np.full((N_CORES, t_total * 128), -1.0, dtype=np.float32)
    idx16[core[order], slot] = idxg[order].astype(np.int16)
    lanef[core[order], slot] = (lane + 128 * sub)[order].astype(np.float32)

    # gather calls: split chunks at group boundaries and the 1024-idx cap
    max_call = 1024 // 128
    tile_grp = np.repeat(np.arange(nseg_tot) // npair, ts)
    chunks = []
    for t0 in range(0, t_total, TC):
        t1 = min(t0 + TC, t_total)
        calls = []
        s = t0
        while s < t1:
            g = int(tile_grp[s])
            e = s
            while e < t1 and tile_grp[e] == g and e - s < max_call:
                e += 1
            calls.append((g, s, e, (e - s) * 128))
            s = e
        chunks.append((t0, t1, calls))

    # segments (in stream order) and per-block last segment
    segs = []  # (pair, t_start, t_end)
    for sid in range(nseg_tot):
        if ts[sid] > 0:
            segs.append(
                (sid % npair, int(seg_t0[sid]), int(seg_t0[sid] + ts[sid]))
            )
    last_end = {}
    nseg = np.zeros(PB, dtype=np.int64)
    for p, s0, s1 in segs:
        last_end[2 * p] = s1
        last_end[2 * p + 1] = s1
        nseg[2 * p] += 1
        nseg[2 * p + 1] += 1
    return {
        "t_total": t_total,
        "chunks": chunks,
        "segs": segs,
        "last_end": last_end,
        "nseg": nseg,
        "idx16": idx16,
        "lanef": lanef,
        "slot_all": slot_all,
    }


def build_nc(layout, ncores=N_CORES):
    t_total = layout["t_total"]
    td = t_total + (t_total & 1)  # even for int32 blob packing
    nc = bacc.Bacc("TRN2", target_bir_lowering=False, num_devices=ncores,
                   dynamic_dma_scratch_size=32768)

    kf = F // 128  # contraction chunks for x @ W
    # Packed constant blob (int32 cols): dstf[td/2] | iota256[128] | deg[PB]
    #                                    | bias[C] | w[kf*C/2]
    cb = td // 2 + 128 + PB + C + kf * C // 2
    xt_in = nc.dram_tensor("xt_sh", [F, NP], bf16, kind="ExternalInput")
    cb_in = nc.dram_tensor("cblob", [128, cb], i32, kind="ExternalInput")
    idx_in = nc.dram_tensor("idxs", [128, t_total * 8], i16, kind="ExternalInput")
    out_t = nc.dram_tensor("out", [NP, C], f32, kind="ExternalOutput")

    with tile.TileContext(nc) as tc, ExitStack() as ctx:
        const = ctx.enter_context(tc.tile_pool(name="const", bufs=1))
        dram = ctx.enter_context(tc.tile_pool(name="dram", bufs=1, space="DRAM"))

        blob = const.tile([128, cb], i32)
        nc.sync.dma_start(out=blob[:], in_=cb_in[:, :])
        o1 = td // 2
        o2 = o1 + 128
        o3 = o2 + PB
        o4 = o3 + C
        dstf = blob[:, 0:o1].bitcast(bf16)           # [128, td]
        iota = blob[:, o1:o2].bitcast(bf16)          # [128, 256]
        deg_t = blob[:, o2:o3].bitcast(f32)          # [128, PB]
        bias_t = blob[:, o3:o4].bitcast(f32)         # [128, C]
        w_bf = blob[:, o4:cb].bitcast(bf16)          # [128, kf*C]

        diss = const.tile([128, PB], f32)   # deg^-1/2
        d2 = const.tile([128, PB], f32)     # deg^-1
        alph = const.tile([128, PB], f32)   # deg^-3/2
        nc.vector.reciprocal(d2[:], deg_t)
        nc.scalar.activation(diss[:], d2[:], AF.Sqrt)
        nc.vector.tensor_mul(alph[:], d2[:], diss[:])

        yself = const.tile([128, PB * C], f32)  # xw * deg^-1/2 (self-loop)
        acc = const.tile([128, PB * C], f32)    # aggregated messages

        y_dup = dram.tile([NP, 2 * C], bf16)
        y_ga = dram.tile([4 * NP, 2 * C], bf16, addr_space="Shared")
        y_gb = dram.tile([4 * NP, 2 * C], bf16, addr_space="Shared")

        # ---- Phase A: xw = x @ W, y = xw * diss -> strided y table ----
        tw = 7
        xt3 = xt_in.ap().rearrange("(k p) n -> p k n", p=128)
        with (
            tc.tile_pool(name="xa", bufs=2) as xa,
            tc.tile_pool(name="psA", bufs=4, space="PSUM") as psa,
            tc.tile_pool(name="ya", bufs=2) as yap,
        ):
            for tg in range(PB // tw):
                xg = xa.tile([128, kf, tw * 128], bf16)
                nc.sync.dma_start(
                    out=xg[:],
                    in_=xt3[:, :, tg * tw * 128:(tg + 1) * tw * 128],
                )
                ybg = yap.tile([128, tw * C], bf16)
                for j in range(tw):
                    t = tg * tw + j
                    ps_xw = psa.tile([128, C], f32, tag="psxw")
                    for k in range(kf):
                        nc.tensor.matmul(
                            ps_xw[:],
                            lhsT=xg[:, k, j * 128:(j + 1) * 128],
                            rhs=w_bf[:, k * C:(k + 1) * C],
                            start=(k == 0), stop=(k == kf - 1),
                        )
                    nc.vector.tensor_scalar_mul(
                        ybg[:, j * C:(j + 1) * C], ps_xw[:], diss[:, t:t + 1]
                    )
                    nc.vector.tensor_scalar_mul(
                        yself[:, t * C:(t + 1) * C], ps_xw[:], diss[:, t:t + 1]
                    )
                nc.sync.dma_start(
                    out=y_dup[tg * tw * 128:(tg + 1) * tw * 128, 0:C].rearrange(
                        "(g p) c -> p g c", p=128
                    ),
                    in_=ybg[:].rearrange("p (g c) -> p g c", c=C),
                )

        # ---- Phase B: replicate y table (split so B overlaps Phase C) ----
        nc.gpsimd.collective_compute(
            "AllGather",
            mybir.AluOpType.bypass,
            replica_groups=[list(range(ncores))],
            ins=[y_dup[0:HALF, :].opt()],
            outs=[y_ga[:].opt()],
        )
        nc.gpsimd.collective_compute(
            "AllGather",
            mybir.AluOpType.bypass,
            replica_groups=[list(range(ncores))],
            ins=[y_dup[HALF:NP, :].opt()],
            outs=[y_gb[:].opt()],
        )
        gsrc = [y_ga[0:GROUP, :], y_ga[GROUP:4 * NP, :],
                y_gb[0:GROUP, :], y_gb[GROUP:4 * NP, :]]

        # ---- Phase C: gather + aggregate + epilogue ----
        segs = layout["segs"]
        last_end = layout["last_end"]
        nseg = layout["nseg"]
        seg_i = 0
        ps_open = {}
        done_blocks = []
        og = None
        og_blocks = []

        with (
            tc.tile_pool(name="idxp", bufs=3) as idxp,
            tc.tile_pool(name="gp", bufs=2) as gp,
            tc.tile_pool(name="ohp", bufs=2) as ohp,
            tc.tile_pool(name="psC", bufs=8, space="PSUM") as psc,
            tc.tile_pool(name="ep", bufs=3) as ep,
            tc.tile_pool(name="ogp", bufs=2) as ogp,
        ):
            def epilogue(b):
                nonlocal og, og_blocks
                v = ep.tile([128, C], f32, tag="v")
                if nseg[b] > 0:
                    nc.vector.tensor_add(
                        v[:], acc[:, b * C:(b + 1) * C],
                        yself[:, b * C:(b + 1) * C],
                    )
                else:
                    nc.scalar.copy(v[:], yself[:, b * C:(b + 1) * C])
                nc.vector.tensor_scalar(
                    v[:], v[:], alph[:, b:b + 1], None,
                    op0=mybir.AluOpType.mult,
                )
                nc.vector.tensor_add(v[:], v[:], bias_t)
                nm = ep.tile([128, 1], f32, tag="nm")
                nc.vector.reduce_max(
                    nm[:], v[:], axis=mybir.AxisListType.X, negate=True
                )
                ex = ep.tile([128, C], f32, tag="ex")
                z = ep.tile([128, 1], f32, tag="z")
                nc.scalar.activation(
                    ex[:], v[:], AF.Exp, bias=nm[:], scale=1.0, accum_out=z[:]
                )
                lz = ep.tile([128, 1], f32, tag="lz")
                nc.scalar.activation(lz[:], z[:], AF.Ln)
                c0 = ep.tile([128, 1], f32, tag="c0")
                nc.vector.tensor_sub(c0[:], nm[:], lz[:])
                if og is None:
                    og = ogp.tile([128, EG * C], f32)
                    og_blocks = []
                oslot = len(og_blocks)
                nc.vector.tensor_scalar_add(
                    og[:, oslot * C:(oslot + 1) * C], v[:], c0[:]
                )
                og_blocks.append(b)
                if len(og_blocks) == EG or b == PB - 1:
                    b0 = og_blocks[0]
                    nb = len(og_blocks)
                    assert og_blocks == list(range(b0, b0 + nb))
                    nc.sync.dma_start(
                        out=out_t[b0 * 128:(b0 + nb) * 128, :].rearrange(
                            "(g p) c -> p g c", p=128
                        ),
                        in_=og[:, 0:nb * C].rearrange("p (g c) -> p g c", c=C),
                    )
                    og = None

            for ci, (t0, t1, calls) in enumerate(layout["chunks"]):
                tcn = t1 - t0
                idxt = idxp.tile([128, tcn * 8], i16)
                nc.sync.dma_start(out=idxt[:], in_=idx_in[:, t0 * 8:t1 * 8])
                gbuf = gp.tile([128, tcn * 128], bf16)
                g3 = gbuf[:].rearrange("p (t e) -> p t e", e=128)
                for (g, ts_, te_, reg) in calls:
                    nc.gpsimd.dma_gather(
                        g3[:, ts_ - t0:te_ - t0, :],
                        gsrc[g],
                        idxt[:, (ts_ - t0) * 8:(te_ - t0) * 8],
                        (te_ - ts_) * 128, reg, 128,
                    )
                oh = ohp.tile([128, tcn * 256], bf16)
                oh3 = oh[:].rearrange("p (t l) -> p t l", l=256)
                d3 = (
                    dstf[:, t0:t1]
                    .rearrange("p (t o) -> p t o", o=1)
                    .to_broadcast([128, tcn, 256])
                )
                i3 = (
                    iota[:]
                    .rearrange("p (o l) -> p o l", o=1)
                    .to_broadcast([128, tcn, 256])
                )
                nc.vector.tensor_tensor(
                    out=oh3, in0=d3, in1=i3, op=mybir.AluOpType.is_equal
                )
                # matmuls for all segment pieces inside this chunk; each
                # tile feeds both blocks of its pair (one-hot halves)
                while seg_i < len(segs) and segs[seg_i][1] < t1:
                    p, s0, s1 = segs[seg_i]
                    if s0 >= t0 and seg_i not in ps_open:
                        ps_open[seg_i] = (
                            psc.tile([128, C], f32, tag="agg",
                                     name=f"aggA{seg_i}"),
                            psc.tile([128, C], f32, tag="agg",
                                     name=f"aggB{seg_i}"),
                        )
                    pss = ps_open[seg_i]
                    for t in range(max(s0, t0), min(s1, t1)):
                        for h in range(2):
                            nc.tensor.matmul(
                                pss[h][:],
                                lhsT=oh3[:, t - t0, h * 128:(h + 1) * 128],
                                rhs=g3[:, t - t0, 0:C],
                                start=(t == s0),
                                stop=(t == s1 - 1),
                            )
                    if s1 > t1:
                        break  # segment continues in next chunk
                    for h in range(2):
                        b = 2 * p + h
                        first = all(bb != b for bb in done_blocks)
                        if first:
                            nc.scalar.copy(
                                acc[:, b * C:(b + 1) * C], pss[h][:]
                            )
                        else:
                            nc.vector.tensor_add(
                                acc[:, b * C:(b + 1) * C], pss[h][:],
                                acc[:, b * C:(b + 1) * C],
                            )
                        done_blocks.append(b)
                    del ps_open[seg_i]
                    if last_end[2 * p] == s1:
                        epilogue(2 * p)
                        epilogue(2 * p + 1)
                    seg_i += 1
            for b in range(PB):
                if nseg[b] == 0:
                    epilogue(b)

    nc.compile()
    return nc


def host_prep(x, edge_index, W, b, layout):
    """Pure index/layout preprocessing. Returns per-core input maps."""
    src = np.asarray(edge_index[0], dtype=np.int64)
    dst = np.asarray(edge_index[1], dtype=np.int64)
    deg = (np.bincount(dst, minlength=N) + 1).astype(np.float32)

    t_total = layout["t_total"]
    td = t_total + (t_total & 1)
    kf = F // 128

    iota_arr = np.broadcast_to(
        np.arange(256, dtype=np.float32), (128, 256)
    ).astype(ml_dtypes.bfloat16).copy()
    bias_rep = np.broadcast_to(
        np.asarray(b, dtype=np.float32), (128, C)
    ).astype(np.float32).copy()
    w_arr = np.ascontiguousarray(
        np.asarray(W, dtype=np.float32)
        .reshape(kf, 128, C)
        .transpose(1, 0, 2)
        .astype(ml_dtypes.bfloat16)
    ).reshape(128, kf * C)
    x_bf = np.asarray(x, dtype=np.float32).astype(ml_dtypes.bfloat16)

    in_maps = []
    for c in range(N_CORES):
        sa = layout["slot_all"][c]
        xt_sh = np.zeros((F, NP), dtype=ml_dtypes.bfloat16)
        xt_sh[:, sa] = x_bf[c * NSH:(c + 1) * NSH].T
        deg_slot = np.ones(NP, dtype=np.float32)
        deg_slot[sa] = deg[c * NSH:(c + 1) * NSH]
        deg_sh = np.ascontiguousarray(deg_slot.reshape(PB, 128).T)

        dstf = np.zeros((128, td), dtype=ml_dtypes.bfloat16)
        lf = layout["lanef"][c].reshape(t_total, 128).T  # [128, t_total]
        dstf[:, :t_total] = lf.astype(ml_dtypes.bfloat16)

        blob = np.concatenate(
            [
                dstf.view(np.uint8),
                iota_arr.view(np.uint8),
                deg_sh.view(np.uint8),
                bias_rep.view(np.uint8),
                w_arr.view(np.uint8),
            ],
            axis=1,
        ).view(np.int32)

        idx = layout["idx16"][c]  # [t_total*128]
        idx_tile = np.tile(
            idx.reshape(-1, 16).T, (8, 1)
        ).astype(np.int16)  # [128, t_total*8]

        in_maps.append({"xt_sh": xt_sh, "cblob": blob, "idxs": idx_tile})
    return in_maps


def run(x, edge_index, W, b, trace=False, **spmd_kwargs):
    layout = build_layout(edge_index)
    in_maps = host_prep(x, edge_index, W, b, layout)
    nc = build_nc(layout)
    res = bass_utils.run_bass_kernel_spmd(
        nc, in_maps, core_ids=list(range(N_CORES)), trace=trace, **spmd_kwargs
    )
    out = np.concatenate(
        [res.results[c]["out"][layout["slot_all"][c]] for c in range(N_CORES)],
        axis=0,
    )
    return out, res


def kernel(x, edge_index, W, b):
    out, _ = run(x, edge_index, W, b)
    return out


# revision 26
# speedup vs baseline: 1.7002x; 1.7002x over previous
"""GCN message-passing kernel for 8 Trainium2 NeuronCores.

out = log_softmax(mean_agg(norm * (x@W)[src] -> dst) + b)

Strategy (graph/data parallel per the sharding hint):
  - Shard dst nodes (and their incoming edges) across 8 cores; within a
    core, dst node i maps to block i//128, lane i%128 (98 blocks).
  - Phase A: each core computes xw = x_shard @ W (PE, bf16), scales by
    deg^-1/2 into y (bf16), stores its y shard to a 256-byte-strided
    DRAM table (row = 128B of y + 128B pad, to satisfy dma_gather's
    256-byte element-size requirement).
  - Phase B: two AllGathers (half-shards) replicate the y table so every
    core has all 100352 rows; the second half overlaps with Phase C.
  - Phase C: edges sorted by (source-range group, dst block) on host.
    A few large dma_gather calls (16 idxs/descriptor, ~1us fixed cost
    each) pull y[src] rows into SBUF; DVE builds one-hot(dst_lane)
    tiles; PE matmuls aggregate each (group, block) segment in PSUM,
    drained into an SBUF f32 accumulator; per-block epilogue applies
    deg^-3/2 scaling, self-loop, bias, and log_softmax.

Math identity used (self-loops make deg >= 1 and cnt == deg):
  out[d] = deg[d]^-3/2 * (sum_{e: dst=d} y[src_e] + y[d]) + b
  with y[n] = xw[n] * deg[n]^-1/2, followed by row log_softmax.
"""

from contextlib import ExitStack

import numpy as np
import ml_dtypes

import concourse.bacc as bacc
import concourse.bass as bass
import concourse.mybir as mybir
import concourse.tile as tile
from concourse import bass_utils

# Problem sizes (hardcoded per the harness contract).
N = 100000
F = 256
C = 64
E = 3200000
N_CORES = 8
NSH = N // N_CORES          # 12500 dst nodes per core
PB = (NSH + 127) // 128     # 98 blocks of 128 dst nodes
NP = PB * 128               # padded shard rows (12544)
HALF = NP // 2              # 6272 rows: half-shard for the split AllGather
GROUP = NP * N_CORES // 4   # 25088 rows per gather group (int16-indexable)
TC = 104                    # gather-chunk size in 128-edge tiles
EG = 7                      # output blocks per store DMA

f32 = mybir.dt.float32
bf16 = mybir.dt.bfloat16
i32 = mybir.dt.int32
i16 = mybir.dt.int16
AF = mybir.ActivationFunctionType


NHH = NSH // 2                  # 6250 nodes per fixed half
HB = PB // 2                    # 49 blocks per half (49*128 = 6272 = HALF)


def _balance(loads, nblk, cap=1016):
    """Greedy node->block assignment keeping per-group in-edge loads under
    cap (so SPMD segments stay at 8 tiles); the last block of the range is
    the designated overflow dump, aligned across cores."""
    order = np.argsort(-loads.sum(1), kind="stable")
    load = np.zeros((nblk, 4), dtype=np.int64)
    fill = np.zeros(nblk, dtype=np.int64)
    slot = np.zeros(len(loads), dtype=np.int64)
    for n in order:
        ln = loads[n]
        newl = load + ln
        over = (newl > cap).sum(axis=1).astype(np.float64)
        over[nblk - 1] = 0.0
        score = over * 1e9 + np.max(newl, axis=1)
        score[fill >= 128] = np.inf
        b = int(np.argmin(score))
        slot[n] = b * 128 + fill[b]
        fill[b] += 1
        load[b] += ln
    return slot


def build_layout(edge_index):
    """Shared host-side layout: edge -> (core, slot) with the tile stream
    sorted by (group, block), segments padded so all cores share one SPMD
    tile map. Returns layout dict + per-core slot fill data."""
    src = np.asarray(edge_index[0], dtype=np.int64)
    dst = np.asarray(edge_index[1], dtype=np.int64)

    core = dst // NSH
    dloc = dst % NSH
    scre = src // NSH
    sloc = src % NSH
    # Fixed half pre-split (local idx // 6250) makes a source's gather
    # group = 2*half + (core >= 4) independent of the within-half block
    # assignment, so block balancing can't shift groups.
    half_s = sloc // NHH
    grp = half_s * 2 + (scre // 4)
    # per-node in-edge loads by source group -> balanced blocks per half
    nl = np.zeros((N, 4), dtype=np.int64)
    np.add.at(nl, (dst, grp), 1)
    slot_all = np.zeros((N_CORES, NSH), dtype=np.int64)
    for c in range(N_CORES):
        cl = nl[c * NSH:(c + 1) * NSH]
        slot_all[c, :NHH] = _balance(cl[:NHH], HB)
        slot_all[c, NHH:] = HALF + _balance(cl[NHH:], HB)

    d_slot = slot_all[core, dloc]
    blk = d_slot // 128
    lane = d_slot % 128
    s_slot = slot_all[scre, sloc]
    p_tab = half_s * (4 * NP) + scre * HALF + (s_slot - half_s * HALF)
    assert (p_tab // GROUP == grp).all()
    idxg = p_tab % GROUP           # int16-safe (< 25088)

    # per (core, group, block) edge counts -> shared segment tile counts
    gb = grp * PB + blk            # 0 .. 4*PB
    cgb = core * (4 * PB) + gb
    cnt = np.bincount(cgb, minlength=N_CORES * 4 * PB).reshape(N_CORES, 4 * PB)
    maxc = cnt.max(axis=0)         # shared (SPMD) real idx count per segment
    ts = (np.ceil(maxc / 128).astype(np.int64))  # tiles per segment
    seg_t0 = np.zeros(4 * PB, dtype=np.int64)
    np.cumsum(ts[:-1], out=seg_t0[1:])
    t_total = int(ts.sum())

    # slot of each edge: segment base + rank within (core, segment)
    order = np.lexsort((gb, core))  # stable by core, then (g, b)
    inv_start = np.zeros(N_CORES * 4 * PB, dtype=np.int64)
    np.cumsum(cnt.reshape(-1)[:-1], out=inv_start[1:])
    rank = np.arange(len(dst), dtype=np.int64) - inv_start[cgb[order]]
    slot = seg_t0[gb[order]] * 128 + rank

    # idx fill: real edges, 0-padding (gathered, masked by zero one-hot)
    idx16 = np.zeros((N_CORES, t_total * 128), dtype=np.int16)
    lanef = np.full((N_CORES, t_total * 128), -1.0, dtype=np.float32)
    idx16[core[order], slot] = idxg[order].astype(np.int16)
    lanef[core[order], slot] = lane[order].astype(np.float32)

    # gather calls: split chunks at group boundaries and the 1024-idx cap
    max_call = 1024 // 128
    tile_grp = np.repeat(np.arange(4 * PB) // PB, ts)
    chunks = []
    for t0 in range(0, t_total, TC):
        t1 = min(t0 + TC, t_total)
        calls = []
        s = t0
        while s < t1:
            g = int(tile_grp[s])
            e = s
            while e < t1 and tile_grp[e] == g and e - s < max_call:
                e += 1
            calls.append((g, s, e, (e - s) * 128))
            s = e
        chunks.append((t0, t1, calls))

    # segments (in stream order) and per-block last segment
    segs = []  # (block, t_start, t_end)
    for sid in range(4 * PB):
        if ts[sid] > 0:
            segs.append((sid % PB, int(seg_t0[sid]), int(seg_t0[sid] + ts[sid])))
    last_end = {}
    nseg = np.zeros(PB, dtype=np.int64)
    for b, s0, s1 in segs:
        last_end[b] = s1
        nseg[b] += 1
    return {
        "t_total": t_total,
        "chunks": chunks,
        "segs": segs,
        "last_end": last_end,
        "nseg": nseg,
        "idx16": idx16,
        "lanef": lanef,
        "slot_all": slot_all,
    }


def build_nc(layout, ncores=N_CORES):
    t_total = layout["t_total"]
    td = t_total + (t_total & 1)  # even for int32 blob packing
    nc = bacc.Bacc("TRN2", target_bir_lowering=False, num_devices=ncores,
                   dynamic_dma_scratch_size=32768, num_swdge_queues=2)

    kf = F // 128  # contraction chunks for x @ W
    # Packed constant blob (int32 cols): dstf[td/2] | iota[64] | deg[PB]
    #                                    | bias[C] | w[kf*C/2]
    cb = td // 2 + 64 + PB + C + kf * C // 2
    xt_in = nc.dram_tensor("xt_sh", [F, NP], bf16, kind="ExternalInput")
    cb_in = nc.dram_tensor("cblob", [128, cb], i32, kind="ExternalInput")
    idx_in = nc.dram_tensor("idxs", [128, t_total * 8], i16, kind="ExternalInput")
    out_t = nc.dram_tensor("out", [NP, C], f32, kind="ExternalOutput")

    with tile.TileContext(nc) as tc, ExitStack() as ctx:
        const = ctx.enter_context(tc.tile_pool(name="const", bufs=1))
        dram = ctx.enter_context(tc.tile_pool(name="dram", bufs=1, space="DRAM"))

        blob = const.tile([128, cb], i32)
        nc.sync.dma_start(out=blob[:], in_=cb_in[:, :])
        o1 = td // 2
        o2 = o1 + 64
        o3 = o2 + PB
        o4 = o3 + C
        dstf = blob[:, 0:o1].bitcast(bf16)           # [128, td]
        iota = blob[:, o1:o2].bitcast(bf16)          # [128, 128]
        deg_t = blob[:, o2:o3].bitcast(f32)          # [128, PB]
        bias_t = blob[:, o3:o4].bitcast(f32)         # [128, C]
        w_bf = blob[:, o4:cb].bitcast(bf16)          # [128, kf*C]

        diss = const.tile([128, PB], f32)   # deg^-1/2
        d2 = const.tile([128, PB], f32)     # deg^-1
        alph = const.tile([128, PB], f32)   # deg^-3/2
        nc.vector.reciprocal(d2[:], deg_t)
        nc.scalar.activation(diss[:], d2[:], AF.Sqrt)
        nc.vector.tensor_mul(alph[:], d2[:], diss[:])

        yself = const.tile([128, PB * C], f32)  # xw * deg^-1/2 (self-loop)
        acc = const.tile([128, PB * C], f32)    # aggregated messages

        y_dup = dram.tile([NP, 2 * C], bf16)
        y_ga = dram.tile([4 * NP, 2 * C], bf16, addr_space="Shared")
        y_gb = dram.tile([4 * NP, 2 * C], bf16, addr_space="Shared")

        # ---- Phase A: xw = x @ W, y = xw * diss -> strided y table ----
        tw = 7
        xt3 = xt_in.ap().rearrange("(k p) n -> p k n", p=128)
        with (
            tc.tile_pool(name="xa", bufs=2) as xa,
            tc.tile_pool(name="psA", bufs=4, space="PSUM") as psa,
            tc.tile_pool(name="ya", bufs=2) as yap,
        ):
            for tg in range(PB // tw):
                xg = xa.tile([128, kf, tw * 128], bf16)
                nc.sync.dma_start(
                    out=xg[:],
                    in_=xt3[:, :, tg * tw * 128:(tg + 1) * tw * 128],
                )
                ybg = yap.tile([128, tw * C], bf16)
                for j in range(tw):
                    t = tg * tw + j
                    ps_xw = psa.tile([128, C], f32, tag="psxw")
                    for k in range(kf):
                        nc.tensor.matmul(
                            ps_xw[:],
                            lhsT=xg[:, k, j * 128:(j + 1) * 128],
                            rhs=w_bf[:, k * C:(k + 1) * C],
                            start=(k == 0), stop=(k == kf - 1),
                        )
                    nc.vector.tensor_scalar_mul(
                        ybg[:, j * C:(j + 1) * C], ps_xw[:], diss[:, t:t + 1]
                    )
                    nc.vector.tensor_scalar_mul(
                        yself[:, t * C:(t + 1) * C], ps_xw[:], diss[:, t:t + 1]
                    )
                nc.sync.dma_start(
                    out=y_dup[tg * tw * 128:(tg + 1) * tw * 128, 0:C].rearrange(
                        "(g p) c -> p g c", p=128
                    ),
                    in_=ybg[:].rearrange("p (g c) -> p g c", c=C),
                )

        # ---- Phase B: replicate y table (split so B overlaps Phase C) ----
        nc.gpsimd.collective_compute(
            "AllGather",
            mybir.AluOpType.bypass,
            replica_groups=[list(range(ncores))],
            ins=[y_dup[0:HALF, :].opt()],
            outs=[y_ga[:].opt()],
        )
        nc.gpsimd.collective_compute(
            "AllGather",
            mybir.AluOpType.bypass,
            replica_groups=[list(range(ncores))],
            ins=[y_dup[HALF:NP, :].opt()],
            outs=[y_gb[:].opt()],
        )
        gsrc = [y_ga[0:GROUP, :], y_ga[GROUP:4 * NP, :],
                y_gb[0:GROUP, :], y_gb[GROUP:4 * NP, :]]

        # ---- Phase C: gather + aggregate + epilogue ----
        segs = layout["segs"]
        last_end = layout["last_end"]
        nseg = layout["nseg"]
        seg_i = 0
        ps_open = {}
        done_blocks = []
        og = None
        og_blocks = []
        call_no = [0]

        with (
            tc.tile_pool(name="idxp", bufs=3) as idxp,
            tc.tile_pool(name="gp", bufs=2) as gp,
            tc.tile_pool(name="ohp", bufs=2) as ohp,
            tc.tile_pool(name="psC", bufs=8, space="PSUM") as psc,
            tc.tile_pool(name="ep", bufs=3) as ep,
            tc.tile_pool(name="ogp", bufs=2) as ogp,
        ):
            def epilogue(b):
                nonlocal og, og_blocks
                v = ep.tile([128, C], f32, tag="v")
                if nseg[b] > 0:
                    nc.vector.tensor_add(
                        v[:], acc[:, b * C:(b + 1) * C],
                        yself[:, b * C:(b + 1) * C],
                    )
                else:
                    nc.scalar.copy(v[:], yself[:, b * C:(b + 1) * C])
                nc.vector.tensor_scalar(
                    v[:], v[:], alph[:, b:b + 1], None,
                    op0=mybir.AluOpType.mult,
                )
                nc.vector.tensor_add(v[:], v[:], bias_t)
                nm = ep.tile([128, 1], f32, tag="nm")
                nc.vector.reduce_max(
                    nm[:], v[:], axis=mybir.AxisListType.X, negate=True
                )
                ex = ep.tile([128, C], f32, tag="ex")
                z = ep.tile([128, 1], f32, tag="z")
                nc.scalar.activation(
                    ex[:], v[:], AF.Exp, bias=nm[:], scale=1.0, accum_out=z[:]
                )
                lz = ep.tile([128, 1], f32, tag="lz")
                nc.scalar.activation(lz[:], z[:], AF.Ln)
                c0 = ep.tile([128, 1], f32, tag="c0")
                nc.vector.tensor_sub(c0[:], nm[:], lz[:])
                if og is None:
                    og = ogp.tile([128, EG * C], f32)
                    og_blocks = []
                oslot = len(og_blocks)
                nc.vector.tensor_scalar_add(
                    og[:, oslot * C:(oslot + 1) * C], v[:], c0[:]
                )
                og_blocks.append(b)
                if len(og_blocks) == EG or b == PB - 1:
                    b0 = og_blocks[0]
                    nb = len(og_blocks)
                    assert og_blocks == list(range(b0, b0 + nb))
                    nc.sync.dma_start(
                        out=out_t[b0 * 128:(b0 + nb) * 128, :].rearrange(
                            "(g p) c -> p g c", p=128
                        ),
                        in_=og[:, 0:nb * C].rearrange("p (g c) -> p g c", c=C),
                    )
                    og = None

            for ci, (t0, t1, calls) in enumerate(layout["chunks"]):
                tcn = t1 - t0
                idxt = idxp.tile([128, tcn * 8], i16)
                nc.sync.dma_start(out=idxt[:], in_=idx_in[:, t0 * 8:t1 * 8])
                gbuf = gp.tile([128, tcn * 128], bf16)
                g3 = gbuf[:].rearrange("p (t e) -> p t e", e=128)
                for (g, ts_, te_, reg) in calls:
                    nc.gpsimd.dma_gather(
                        g3[:, ts_ - t0:te_ - t0, :],
                        gsrc[g],
                        idxt[:, (ts_ - t0) * 8:(te_ - t0) * 8],
                        (te_ - ts_) * 128, reg, 128,
                        queue_num=call_no[0] % 2,
                    )
                    call_no[0] += 1
                oh = ohp.tile([128, tcn * 128], bf16)
                oh3 = oh[:].rearrange("p (t l) -> p t l", l=128)
                d3 = (
                    dstf[:, t0:t1]
                    .rearrange("p (t o) -> p t o", o=1)
                    .to_broadcast([128, tcn, 128])
                )
                i3 = (
                    iota[:]
                    .rearrange("p (o l) -> p o l", o=1)
                    .to_broadcast([128, tcn, 128])
                )
                nc.vector.tensor_tensor(
                    out=oh3, in0=d3, in1=i3, op=mybir.AluOpType.is_equal
                )
                # matmuls for all segment pieces inside this chunk
                while seg_i < len(segs) and segs[seg_i][1] < t1:
                    b, s0, s1 = segs[seg_i]
                    if s0 >= t0 and seg_i not in ps_open:
                        ps_open[seg_i] = psc.tile(
                            [128, C], f32, tag="agg", name=f"agg{seg_i}"
                        )
                    pss = ps_open[seg_i]
                    for t in range(max(s0, t0), min(s1, t1)):
                        nc.tensor.matmul(
                            pss[:],
                            lhsT=oh3[:, t - t0, :],
                            rhs=g3[:, t - t0, 0:C],
                            start=(t == s0),
                            stop=(t == s1 - 1),
                        )
                    if s1 > t1:
                        break  # segment continues in next chunk
                    # drain psum into acc
                    first = not done_blocks or all(
                        bb != b for bb in done_blocks
                    )
                    if first:
                        nc.scalar.copy(acc[:, b * C:(b + 1) * C], pss[:])
                    else:
                        nc.vector.tensor_add(
                            acc[:, b * C:(b + 1) * C], pss[:],
                            acc[:, b * C:(b + 1) * C],
                        )
                    done_blocks.append(b)
                    del ps_open[seg_i]
                    if last_end[b] == s1:
                        epilogue(b)
                    seg_i += 1
            for b in range(PB):
                if nseg[b] == 0:
                    epilogue(b)

    nc.compile()
    return nc


def host_prep(x, edge_index, W, b, layout):
    """Pure index/layout preprocessing. Returns per-core input maps."""
    src = np.asarray(edge_index[0], dtype=np.int64)
    dst = np.asarray(edge_index[1], dtype=np.int64)
    deg = (np.bincount(dst, minlength=N) + 1).astype(np.float32)

    t_total = layout["t_total"]
    td = t_total + (t_total & 1)
    kf = F // 128

    iota_arr = np.broadcast_to(
        np.arange(128, dtype=np.float32), (128, 128)
    ).astype(ml_dtypes.bfloat16).copy()
    bias_rep = np.broadcast_to(
        np.asarray(b, dtype=np.float32), (128, C)
    ).astype(np.float32).copy()
    w_arr = np.ascontiguousarray(
        np.asarray(W, dtype=np.float32)
        .reshape(kf, 128, C)
        .transpose(1, 0, 2)
        .astype(ml_dtypes.bfloat16)
    ).reshape(128, kf * C)
    x_bf = np.asarray(x, dtype=np.float32).astype(ml_dtypes.bfloat16)

    in_maps = []
    for c in range(N_CORES):
        sa = layout["slot_all"][c]
        xt_sh = np.zeros((F, NP), dtype=ml_dtypes.bfloat16)
        xt_sh[:, sa] = x_bf[c * NSH:(c + 1) * NSH].T
        deg_slot = np.ones(NP, dtype=np.float32)
        deg_slot[sa] = deg[c * NSH:(c + 1) * NSH]
        deg_sh = np.ascontiguousarray(deg_slot.reshape(PB, 128).T)

        dstf = np.zeros((128, td), dtype=ml_dtypes.bfloat16)
        lf = layout["lanef"][c].reshape(t_total, 128).T  # [128, t_total]
        dstf[:, :t_total] = lf.astype(ml_dtypes.bfloat16)

        blob = np.concatenate(
            [
                dstf.view(np.uint8),
                iota_arr.view(np.uint8),
                deg_sh.view(np.uint8),
                bias_rep.view(np.uint8),
                w_arr.view(np.uint8),
            ],
            axis=1,
        ).view(np.int32)

        idx = layout["idx16"][c]  # [t_total*128]
        idx_tile = np.tile(
            idx.reshape(-1, 16).T, (8, 1)
        ).astype(np.int16)  # [128, t_total*8]

        in_maps.append({"xt_sh": xt_sh, "cblob": blob, "idxs": idx_tile})
    return in_maps


def run(x, edge_index, W, b, trace=False, **spmd_kwargs):
    layout = build_layout(edge_index)
    in_maps = host_prep(x, edge_index, W, b, layout)
    nc = build_nc(layout)
    res = bass_utils.run_bass_kernel_spmd(
        nc, in_maps, core_ids=list(range(N_CORES)), trace=trace, **spmd_kwargs
    )
    out = np.concatenate(
        [res.results[c]["out"][layout["slot_all"][c]] for c in range(N_CORES)],
        axis=0,
    )
    return out, res


def kernel(x, edge_index, W, b):
    out, _ = run(x, edge_index, W, b)
    return out


# revision 27
# speedup vs baseline: 2.2318x; 1.3126x over previous
"""GCN message-passing kernel for 8 Trainium2 NeuronCores.

out = log_softmax(mean_agg(norm * (x@W)[src] -> dst) + b)

Strategy (graph/data parallel per the sharding hint):
  - Shard dst nodes (and their incoming edges) across 8 cores; within a
    core, dst node i maps to block i//128, lane i%128 (98 blocks).
  - Phase A: each core computes xw = x_shard @ W (PE, bf16), scales by
    deg^-1/2 into y (bf16), stores its y shard to a 256-byte-strided
    DRAM table (row = 128B of y + 128B pad, to satisfy dma_gather's
    256-byte element-size requirement).
  - Phase B: two AllGathers (half-shards) replicate the y table so every
    core has all 100352 rows; the second half overlaps with Phase C.
  - Phase C: edges sorted by (source-range group, dst block) on host.
    A few large dma_gather calls (16 idxs/descriptor, ~1us fixed cost
    each) pull y[src] rows into SBUF; DVE builds one-hot(dst_lane)
    tiles; PE matmuls aggregate each (group, block) segment in PSUM,
    drained into an SBUF f32 accumulator; per-block epilogue applies
    deg^-3/2 scaling, self-loop, bias, and log_softmax.

Math identity used (self-loops make deg >= 1 and cnt == deg):
  out[d] = deg[d]^-3/2 * (sum_{e: dst=d} y[src_e] + y[d]) + b
  with y[n] = xw[n] * deg[n]^-1/2, followed by row log_softmax.
"""

from contextlib import ExitStack

import numpy as np
import ml_dtypes

import concourse.bacc as bacc
import concourse.bass as bass
import concourse.mybir as mybir
import concourse.tile as tile
from concourse import bass_utils

# Problem sizes (hardcoded per the harness contract).
N = 100000
F = 256
C = 64
E = 3200000
N_CORES = 8
NSH = N // N_CORES          # 12500 dst nodes per core
PB = (NSH + 127) // 128     # 98 blocks of 128 dst nodes
NP = PB * 128               # padded shard rows (12544)
HALF = NP // 2              # 6272 rows: half-shard for the split AllGather
GROUP = NP * N_CORES // 4   # 25088 rows per gather group (int16-indexable)
TC = 104                    # gather-chunk size in 128-edge tiles
EG = 7                      # output blocks per store DMA

f32 = mybir.dt.float32
bf16 = mybir.dt.bfloat16
i32 = mybir.dt.int32
i16 = mybir.dt.int16
AF = mybir.ActivationFunctionType


NHH = NSH // 2                  # 6250 nodes per fixed half
HB = PB // 2                    # 49 blocks per half (49*128 = 6272 = HALF)


def _balance(loads, nblk, cap=1016):
    """Greedy node->block assignment keeping per-group in-edge loads under
    cap (so SPMD segments stay at 8 tiles); the last block of the range is
    the designated overflow dump, aligned across cores."""
    order = np.argsort(-loads.sum(1), kind="stable")
    load = np.zeros((nblk, 4), dtype=np.int64)
    fill = np.zeros(nblk, dtype=np.int64)
    slot = np.zeros(len(loads), dtype=np.int64)
    for n in order:
        ln = loads[n]
        newl = load + ln
        over = (newl > cap).sum(axis=1).astype(np.float64)
        over[nblk - 1] = 0.0
        score = over * 1e9 + np.max(newl, axis=1)
        score[fill >= 128] = np.inf
        b = int(np.argmin(score))
        slot[n] = b * 128 + fill[b]
        fill[b] += 1
        load[b] += ln
    return slot


def build_layout(edge_index):
    """Shared host-side layout: edge -> (core, slot) with the tile stream
    sorted by (group, block), segments padded so all cores share one SPMD
    tile map. Returns layout dict + per-core slot fill data."""
    src = np.asarray(edge_index[0], dtype=np.int64)
    dst = np.asarray(edge_index[1], dtype=np.int64)

    core = dst // NSH
    dloc = dst % NSH
    scre = src // NSH
    sloc = src % NSH
    # Fixed half pre-split (local idx // 6250) makes a source's gather
    # group = 2*half + (core >= 4) independent of the within-half block
    # assignment, so block balancing can't shift groups.
    half_s = sloc // NHH
    grp = half_s * 2 + (scre // 4)
    # per-node in-edge loads by source group -> balanced blocks per half
    nl = np.zeros((N, 4), dtype=np.int64)
    np.add.at(nl, (dst, grp), 1)
    slot_all = np.zeros((N_CORES, NSH), dtype=np.int64)
    for c in range(N_CORES):
        cl = nl[c * NSH:(c + 1) * NSH]
        slot_all[c, :NHH] = _balance(cl[:NHH], HB)
        slot_all[c, NHH:] = HALF + _balance(cl[NHH:], HB)

    d_slot = slot_all[core, dloc]
    blk = d_slot // 128
    lane = d_slot % 128
    s_slot = slot_all[scre, sloc]
    p_tab = half_s * (4 * NP) + scre * HALF + (s_slot - half_s * HALF)
    assert (p_tab // GROUP == grp).all()
    idxg = p_tab % GROUP           # int16-safe (< 25088)

    # per (core, group, block) edge counts -> shared segment tile counts
    gb = grp * PB + blk            # 0 .. 4*PB
    cgb = core * (4 * PB) + gb
    cnt = np.bincount(cgb, minlength=N_CORES * 4 * PB).reshape(N_CORES, 4 * PB)
    maxc = cnt.max(axis=0)         # shared (SPMD) real idx count per segment
    ts = (np.ceil(maxc / 128).astype(np.int64))  # tiles per segment
    seg_t0 = np.zeros(4 * PB, dtype=np.int64)
    np.cumsum(ts[:-1], out=seg_t0[1:])
    t_total = int(ts.sum())

    # slot of each edge: segment base + rank within (core, segment)
    order = np.lexsort((gb, core))  # stable by core, then (g, b)
    inv_start = np.zeros(N_CORES * 4 * PB, dtype=np.int64)
    np.cumsum(cnt.reshape(-1)[:-1], out=inv_start[1:])
    rank = np.arange(len(dst), dtype=np.int64) - inv_start[cgb[order]]
    slot = seg_t0[gb[order]] * 128 + rank

    # idx fill: real edges, 0-padding (gathered, masked by zero one-hot)
    idx16 = np.zeros((N_CORES, t_total * 128), dtype=np.int16)
    lanef = np.full((N_CORES, t_total * 128), -1.0, dtype=np.float32)
    idx16[core[order], slot] = idxg[order].astype(np.int16)
    lanef[core[order], slot] = lane[order].astype(np.float32)

    # gather calls: split chunks at group boundaries and the 1024-idx cap
    max_call = 1024 // 128
    tile_grp = np.repeat(np.arange(4 * PB) // PB, ts)
    chunks = []
    for t0 in range(0, t_total, TC):
        t1 = min(t0 + TC, t_total)
        calls = []
        s = t0
        while s < t1:
            g = int(tile_grp[s])
            e = s
            while e < t1 and tile_grp[e] == g and e - s < max_call:
                e += 1
            calls.append((g, s, e, (e - s) * 128))
            s = e
        chunks.append((t0, t1, calls))

    # segments (in stream order) and per-block last segment
    segs = []  # (block, t_start, t_end)
    for sid in range(4 * PB):
        if ts[sid] > 0:
            segs.append((sid % PB, int(seg_t0[sid]), int(seg_t0[sid] + ts[sid])))
    last_end = {}
    nseg = np.zeros(PB, dtype=np.int64)
    for b, s0, s1 in segs:
        last_end[b] = s1
        nseg[b] += 1
    return {
        "t_total": t_total,
        "chunks": chunks,
        "segs": segs,
        "last_end": last_end,
        "nseg": nseg,
        "idx16": idx16,
        "lanef": lanef,
        "slot_all": slot_all,
    }


def build_nc(layout, ncores=N_CORES):
    t_total = layout["t_total"]
    td = t_total + (t_total & 1)  # even for int32 blob packing
    nc = bacc.Bacc("TRN2", target_bir_lowering=False, num_devices=ncores,
                   dynamic_dma_scratch_size=32768, num_swdge_queues=4)

    kf = F // 128  # contraction chunks for x @ W
    # Packed constant blob (int32 cols): dstf[td/2] | iota[64] | deg[PB]
    #                                    | bias[C] | w[kf*C/2]
    cb = td // 2 + 64 + PB + C + kf * C // 2
    xt_in = nc.dram_tensor("xt_sh", [F, NP], bf16, kind="ExternalInput")
    cb_in = nc.dram_tensor("cblob", [128, cb], i32, kind="ExternalInput")
    idx_in = nc.dram_tensor("idxs", [128, t_total * 8], i16, kind="ExternalInput")
    out_t = nc.dram_tensor("out", [NP, C], f32, kind="ExternalOutput")

    with tile.TileContext(nc) as tc, ExitStack() as ctx:
        const = ctx.enter_context(tc.tile_pool(name="const", bufs=1))
        dram = ctx.enter_context(tc.tile_pool(name="dram", bufs=1, space="DRAM"))

        blob = const.tile([128, cb], i32)
        nc.sync.dma_start(out=blob[:], in_=cb_in[:, :])
        o1 = td // 2
        o2 = o1 + 64
        o3 = o2 + PB
        o4 = o3 + C
        dstf = blob[:, 0:o1].bitcast(bf16)           # [128, td]
        iota = blob[:, o1:o2].bitcast(bf16)          # [128, 128]
        deg_t = blob[:, o2:o3].bitcast(f32)          # [128, PB]
        bias_t = blob[:, o3:o4].bitcast(f32)         # [128, C]
        w_bf = blob[:, o4:cb].bitcast(bf16)          # [128, kf*C]

        diss = const.tile([128, PB], f32)   # deg^-1/2
        d2 = const.tile([128, PB], f32)     # deg^-1
        alph = const.tile([128, PB], f32)   # deg^-3/2
        nc.vector.reciprocal(d2[:], deg_t)
        nc.scalar.activation(diss[:], d2[:], AF.Sqrt)
        nc.vector.tensor_mul(alph[:], d2[:], diss[:])

        yself = const.tile([128, PB * C], f32)  # xw * deg^-1/2 (self-loop)
        acc = const.tile([128, PB * C], f32)    # aggregated messages

        y_dup = dram.tile([NP, 2 * C], bf16)
        y_ga = dram.tile([4 * NP, 2 * C], bf16, addr_space="Shared")
        y_gb = dram.tile([4 * NP, 2 * C], bf16, addr_space="Shared")

        # ---- Phase A: xw = x @ W, y = xw * diss -> strided y table ----
        tw = 7
        xt3 = xt_in.ap().rearrange("(k p) n -> p k n", p=128)
        with (
            tc.tile_pool(name="xa", bufs=2) as xa,
            tc.tile_pool(name="psA", bufs=4, space="PSUM") as psa,
            tc.tile_pool(name="ya", bufs=2) as yap,
        ):
            for tg in range(PB // tw):
                xg = xa.tile([128, kf, tw * 128], bf16)
                nc.sync.dma_start(
                    out=xg[:],
                    in_=xt3[:, :, tg * tw * 128:(tg + 1) * tw * 128],
                )
                ybg = yap.tile([128, tw * C], bf16)
                for j in range(tw):
                    t = tg * tw + j
                    ps_xw = psa.tile([128, C], f32, tag="psxw")
                    for k in range(kf):
                        nc.tensor.matmul(
                            ps_xw[:],
                            lhsT=xg[:, k, j * 128:(j + 1) * 128],
                            rhs=w_bf[:, k * C:(k + 1) * C],
                            start=(k == 0), stop=(k == kf - 1),
                        )
                    nc.vector.tensor_scalar_mul(
                        ybg[:, j * C:(j + 1) * C], ps_xw[:], diss[:, t:t + 1]
                    )
                    nc.vector.tensor_scalar_mul(
                        yself[:, t * C:(t + 1) * C], ps_xw[:], diss[:, t:t + 1]
                    )
                nc.sync.dma_start(
                    out=y_dup[tg * tw * 128:(tg + 1) * tw * 128, 0:C].rearrange(
                        "(g p) c -> p g c", p=128
                    ),
                    in_=ybg[:].rearrange("p (g c) -> p g c", c=C),
                )

        # ---- Phase B: replicate y table (split so B overlaps Phase C) ----
        nc.gpsimd.collective_compute(
            "AllGather",
            mybir.AluOpType.bypass,
            replica_groups=[list(range(ncores))],
            ins=[y_dup[0:HALF, :].opt()],
            outs=[y_ga[:].opt()],
        )
        nc.gpsimd.collective_compute(
            "AllGather",
            mybir.AluOpType.bypass,
            replica_groups=[list(range(ncores))],
            ins=[y_dup[HALF:NP, :].opt()],
            outs=[y_gb[:].opt()],
        )
        gsrc = [y_ga[0:GROUP, :], y_ga[GROUP:4 * NP, :],
                y_gb[0:GROUP, :], y_gb[GROUP:4 * NP, :]]

        # ---- Phase C: gather + aggregate + epilogue ----
        segs = layout["segs"]
        last_end = layout["last_end"]
        nseg = layout["nseg"]
        seg_i = 0
        ps_open = {}
        done_blocks = []
        og = None
        og_blocks = []
        call_no = [0]

        with (
            tc.tile_pool(name="idxp", bufs=3) as idxp,
            tc.tile_pool(name="gp", bufs=2) as gp,
            tc.tile_pool(name="ohp", bufs=2) as ohp,
            tc.tile_pool(name="psC", bufs=8, space="PSUM") as psc,
            tc.tile_pool(name="ep", bufs=3) as ep,
            tc.tile_pool(name="ogp", bufs=2) as ogp,
        ):
            def epilogue(b):
                nonlocal og, og_blocks
                v = ep.tile([128, C], f32, tag="v")
                if nseg[b] > 0:
                    nc.vector.tensor_add(
                        v[:], acc[:, b * C:(b + 1) * C],
                        yself[:, b * C:(b + 1) * C],
                    )
                else:
                    nc.scalar.copy(v[:], yself[:, b * C:(b + 1) * C])
                nc.vector.tensor_scalar(
                    v[:], v[:], alph[:, b:b + 1], None,
                    op0=mybir.AluOpType.mult,
                )
                nc.vector.tensor_add(v[:], v[:], bias_t)
                nm = ep.tile([128, 1], f32, tag="nm")
                nc.vector.reduce_max(
                    nm[:], v[:], axis=mybir.AxisListType.X, negate=True
                )
                ex = ep.tile([128, C], f32, tag="ex")
                z = ep.tile([128, 1], f32, tag="z")
                nc.scalar.activation(
                    ex[:], v[:], AF.Exp, bias=nm[:], scale=1.0, accum_out=z[:]
                )
                lz = ep.tile([128, 1], f32, tag="lz")
                nc.scalar.activation(lz[:], z[:], AF.Ln)
                c0 = ep.tile([128, 1], f32, tag="c0")
                nc.vector.tensor_sub(c0[:], nm[:], lz[:])
                if og is None:
                    og = ogp.tile([128, EG * C], f32)
                    og_blocks = []
                oslot = len(og_blocks)
                nc.vector.tensor_scalar_add(
                    og[:, oslot * C:(oslot + 1) * C], v[:], c0[:]
                )
                og_blocks.append(b)
                if len(og_blocks) == EG or b == PB - 1:
                    b0 = og_blocks[0]
                    nb = len(og_blocks)
                    assert og_blocks == list(range(b0, b0 + nb))
                    nc.sync.dma_start(
                        out=out_t[b0 * 128:(b0 + nb) * 128, :].rearrange(
                            "(g p) c -> p g c", p=128
                        ),
                        in_=og[:, 0:nb * C].rearrange("p (g c) -> p g c", c=C),
                    )
                    og = None

            for ci, (t0, t1, calls) in enumerate(layout["chunks"]):
                tcn = t1 - t0
                idxt = idxp.tile([128, tcn * 8], i16)
                nc.sync.dma_start(out=idxt[:], in_=idx_in[:, t0 * 8:t1 * 8])
                gbuf = gp.tile([128, tcn * 128], bf16)
                g3 = gbuf[:].rearrange("p (t e) -> p t e", e=128)
                for (g, ts_, te_, reg) in calls:
                    nc.gpsimd.dma_gather(
                        g3[:, ts_ - t0:te_ - t0, :],
                        gsrc[g],
                        idxt[:, (ts_ - t0) * 8:(te_ - t0) * 8],
                        (te_ - ts_) * 128, reg, 128,
                        queue_num=call_no[0] % 4,
                    )
                    call_no[0] += 1
                oh = ohp.tile([128, tcn * 128], bf16)
                oh3 = oh[:].rearrange("p (t l) -> p t l", l=128)
                d3 = (
                    dstf[:, t0:t1]
                    .rearrange("p (t o) -> p t o", o=1)
                    .to_broadcast([128, tcn, 128])
                )
                i3 = (
                    iota[:]
                    .rearrange("p (o l) -> p o l", o=1)
                    .to_broadcast([128, tcn, 128])
                )
                nc.vector.tensor_tensor(
                    out=oh3, in0=d3, in1=i3, op=mybir.AluOpType.is_equal
                )
                # matmuls for all segment pieces inside this chunk
                while seg_i < len(segs) and segs[seg_i][1] < t1:
                    b, s0, s1 = segs[seg_i]
                    if s0 >= t0 and seg_i not in ps_open:
                        ps_open[seg_i] = psc.tile(
                            [128, C], f32, tag="agg", name=f"agg{seg_i}"
                        )
                    pss = ps_open[seg_i]
                    for t in range(max(s0, t0), min(s1, t1)):
                        nc.tensor.matmul(
                            pss[:],
                            lhsT=oh3[:, t - t0, :],
                            rhs=g3[:, t - t0, 0:C],
                            start=(t == s0),
                            stop=(t == s1 - 1),
                        )
                    if s1 > t1:
                        break  # segment continues in next chunk
                    # drain psum into acc
                    first = not done_blocks or all(
                        bb != b for bb in done_blocks
                    )
                    if first:
                        nc.scalar.copy(acc[:, b * C:(b + 1) * C], pss[:])
                    else:
                        nc.vector.tensor_add(
                            acc[:, b * C:(b + 1) * C], pss[:],
                            acc[:, b * C:(b + 1) * C],
                        )
                    done_blocks.append(b)
                    del ps_open[seg_i]
                    if last_end[b] == s1:
                        epilogue(b)
                    seg_i += 1
            for b in range(PB):
                if nseg[b] == 0:
                    epilogue(b)

    nc.compile()
    return nc


def host_prep(x, edge_index, W, b, layout):
    """Pure index/layout preprocessing. Returns per-core input maps."""
    src = np.asarray(edge_index[0], dtype=np.int64)
    dst = np.asarray(edge_index[1], dtype=np.int64)
    deg = (np.bincount(dst, minlength=N) + 1).astype(np.float32)

    t_total = layout["t_total"]
    td = t_total + (t_total & 1)
    kf = F // 128

    iota_arr = np.broadcast_to(
        np.arange(128, dtype=np.float32), (128, 128)
    ).astype(ml_dtypes.bfloat16).copy()
    bias_rep = np.broadcast_to(
        np.asarray(b, dtype=np.float32), (128, C)
    ).astype(np.float32).copy()
    w_arr = np.ascontiguousarray(
        np.asarray(W, dtype=np.float32)
        .reshape(kf, 128, C)
        .transpose(1, 0, 2)
        .astype(ml_dtypes.bfloat16)
    ).reshape(128, kf * C)
    x_bf = np.asarray(x, dtype=np.float32).astype(ml_dtypes.bfloat16)

    in_maps = []
    for c in range(N_CORES):
        sa = layout["slot_all"][c]
        xt_sh = np.zeros((F, NP), dtype=ml_dtypes.bfloat16)
        xt_sh[:, sa] = x_bf[c * NSH:(c + 1) * NSH].T
        deg_slot = np.ones(NP, dtype=np.float32)
        deg_slot[sa] = deg[c * NSH:(c + 1) * NSH]
        deg_sh = np.ascontiguousarray(deg_slot.reshape(PB, 128).T)

        dstf = np.zeros((128, td), dtype=ml_dtypes.bfloat16)
        lf = layout["lanef"][c].reshape(t_total, 128).T  # [128, t_total]
        dstf[:, :t_total] = lf.astype(ml_dtypes.bfloat16)

        blob = np.concatenate(
            [
                dstf.view(np.uint8),
                iota_arr.view(np.uint8),
                deg_sh.view(np.uint8),
                bias_rep.view(np.uint8),
                w_arr.view(np.uint8),
            ],
            axis=1,
        ).view(np.int32)

        idx = layout["idx16"][c]  # [t_total*128]
        idx_tile = np.tile(
            idx.reshape(-1, 16).T, (8, 1)
        ).astype(np.int16)  # [128, t_total*8]

        in_maps.append({"xt_sh": xt_sh, "cblob": blob, "idxs": idx_tile})
    return in_maps


def run(x, edge_index, W, b, trace=False, **spmd_kwargs):
    layout = build_layout(edge_index)
    in_maps = host_prep(x, edge_index, W, b, layout)
    nc = build_nc(layout)
    res = bass_utils.run_bass_kernel_spmd(
        nc, in_maps, core_ids=list(range(N_CORES)), trace=trace, **spmd_kwargs
    )
    out = np.concatenate(
        [res.results[c]["out"][layout["slot_all"][c]] for c in range(N_CORES)],
        axis=0,
    )
    return out, res


def kernel(x, edge_index, W, b):
    out, _ = run(x, edge_index, W, b)
    return out


# revision 28
# speedup vs baseline: 2.2877x; 1.0250x over previous
"""GCN message-passing kernel for 8 Trainium2 NeuronCores.

out = log_softmax(mean_agg(norm * (x@W)[src] -> dst) + b)

Strategy (graph/data parallel per the sharding hint):
  - Shard dst nodes (and their incoming edges) across 8 cores; within a
    core, dst node i maps to block i//128, lane i%128 (98 blocks).
  - Phase A: each core computes xw = x_shard @ W (PE, bf16), scales by
    deg^-1/2 into y (bf16), stores its y shard to a 256-byte-strided
    DRAM table (row = 128B of y + 128B pad, to satisfy dma_gather's
    256-byte element-size requirement).
  - Phase B: two AllGathers (half-shards) replicate the y table so every
    core has all 100352 rows; the second half overlaps with Phase C.
  - Phase C: edges sorted by (source-range group, dst block) on host.
    A few large dma_gather calls (16 idxs/descriptor, ~1us fixed cost
    each) pull y[src] rows into SBUF; DVE builds one-hot(dst_lane)
    tiles; PE matmuls aggregate each (group, block) segment in PSUM,
    drained into an SBUF f32 accumulator; per-block epilogue applies
    deg^-3/2 scaling, self-loop, bias, and log_softmax.

Math identity used (self-loops make deg >= 1 and cnt == deg):
  out[d] = deg[d]^-3/2 * (sum_{e: dst=d} y[src_e] + y[d]) + b
  with y[n] = xw[n] * deg[n]^-1/2, followed by row log_softmax.
"""

from contextlib import ExitStack

import numpy as np
import ml_dtypes

import concourse.bacc as bacc
import concourse.bass as bass
import concourse.mybir as mybir
import concourse.tile as tile
from concourse import bass_utils

# Problem sizes (hardcoded per the harness contract).
N = 100000
F = 256
C = 64
E = 3200000
N_CORES = 8
NSH = N // N_CORES          # 12500 dst nodes per core
PB = (NSH + 127) // 128     # 98 blocks of 128 dst nodes
NP = PB * 128               # padded shard rows (12544)
HALF = NP // 2              # 6272 rows: half-shard for the split AllGather
GROUP = NP * N_CORES // 4   # 25088 rows per gather group (int16-indexable)
TC = 72                     # gather-chunk size in 128-edge tiles
EG = 7                      # output blocks per store DMA

f32 = mybir.dt.float32
bf16 = mybir.dt.bfloat16
i32 = mybir.dt.int32
i16 = mybir.dt.int16
AF = mybir.ActivationFunctionType


NHH = NSH // 2                  # 6250 nodes per fixed half
HB = PB // 2                    # 49 blocks per half (49*128 = 6272 = HALF)


def _balance(loads, nblk, cap=1016):
    """Greedy node->block assignment keeping per-group in-edge loads under
    cap (so SPMD segments stay at 8 tiles); the last block of the range is
    the designated overflow dump, aligned across cores."""
    order = np.argsort(-loads.sum(1), kind="stable")
    load = np.zeros((nblk, 4), dtype=np.int64)
    fill = np.zeros(nblk, dtype=np.int64)
    slot = np.zeros(len(loads), dtype=np.int64)
    for n in order:
        ln = loads[n]
        newl = load + ln
        over = (newl > cap).sum(axis=1).astype(np.float64)
        over[nblk - 1] = 0.0
        score = over * 1e9 + np.max(newl, axis=1)
        score[fill >= 128] = np.inf
        b = int(np.argmin(score))
        slot[n] = b * 128 + fill[b]
        fill[b] += 1
        load[b] += ln
    return slot


def build_layout(edge_index):
    """Shared host-side layout: edge -> (core, slot) with the tile stream
    sorted by (group, block), segments padded so all cores share one SPMD
    tile map. Returns layout dict + per-core slot fill data."""
    src = np.asarray(edge_index[0], dtype=np.int64)
    dst = np.asarray(edge_index[1], dtype=np.int64)

    core = dst // NSH
    dloc = dst % NSH
    scre = src // NSH
    sloc = src % NSH
    # Fixed half pre-split (local idx // 6250) makes a source's gather
    # group = 2*half + (core >= 4) independent of the within-half block
    # assignment, so block balancing can't shift groups.
    half_s = sloc // NHH
    grp = half_s * 2 + (scre // 4)
    # per-node in-edge loads by source group -> balanced blocks per half
    nl = np.zeros((N, 4), dtype=np.int64)
    np.add.at(nl, (dst, grp), 1)
    slot_all = np.zeros((N_CORES, NSH), dtype=np.int64)
    for c in range(N_CORES):
        cl = nl[c * NSH:(c + 1) * NSH]
        slot_all[c, :NHH] = _balance(cl[:NHH], HB)
        slot_all[c, NHH:] = HALF + _balance(cl[NHH:], HB)

    d_slot = slot_all[core, dloc]
    blk = d_slot // 128
    lane = d_slot % 128
    s_slot = slot_all[scre, sloc]
    p_tab = half_s * (4 * NP) + scre * HALF + (s_slot - half_s * HALF)
    assert (p_tab // GROUP == grp).all()
    idxg = p_tab % GROUP           # int16-safe (< 25088)

    # per (core, group, block) edge counts -> shared segment tile counts
    gb = grp * PB + blk            # 0 .. 4*PB
    cgb = core * (4 * PB) + gb
    cnt = np.bincount(cgb, minlength=N_CORES * 4 * PB).reshape(N_CORES, 4 * PB)
    maxc = cnt.max(axis=0)         # shared (SPMD) real idx count per segment
    ts = (np.ceil(maxc / 128).astype(np.int64))  # tiles per segment
    seg_t0 = np.zeros(4 * PB, dtype=np.int64)
    np.cumsum(ts[:-1], out=seg_t0[1:])
    t_total = int(ts.sum())

    # slot of each edge: segment base + rank within (core, segment)
    order = np.lexsort((gb, core))  # stable by core, then (g, b)
    inv_start = np.zeros(N_CORES * 4 * PB, dtype=np.int64)
    np.cumsum(cnt.reshape(-1)[:-1], out=inv_start[1:])
    rank = np.arange(len(dst), dtype=np.int64) - inv_start[cgb[order]]
    slot = seg_t0[gb[order]] * 128 + rank

    # idx fill: real edges, 0-padding (gathered, masked by zero one-hot)
    idx16 = np.zeros((N_CORES, t_total * 128), dtype=np.int16)
    lanef = np.full((N_CORES, t_total * 128), -1.0, dtype=np.float32)
    idx16[core[order], slot] = idxg[order].astype(np.int16)
    lanef[core[order], slot] = lane[order].astype(np.float32)

    # gather calls: split chunks at group boundaries and the 1024-idx cap
    max_call = 1024 // 128
    tile_grp = np.repeat(np.arange(4 * PB) // PB, ts)
    chunks = []
    for t0 in range(0, t_total, TC):
        t1 = min(t0 + TC, t_total)
        calls = []
        s = t0
        while s < t1:
            g = int(tile_grp[s])
            e = s
            while e < t1 and tile_grp[e] == g and e - s < max_call:
                e += 1
            calls.append((g, s, e, (e - s) * 128))
            s = e
        chunks.append((t0, t1, calls))

    # segments (in stream order) and per-block last segment
    segs = []  # (block, t_start, t_end)
    for sid in range(4 * PB):
        if ts[sid] > 0:
            segs.append((sid % PB, int(seg_t0[sid]), int(seg_t0[sid] + ts[sid])))
    last_end = {}
    nseg = np.zeros(PB, dtype=np.int64)
    for b, s0, s1 in segs:
        last_end[b] = s1
        nseg[b] += 1
    return {
        "t_total": t_total,
        "chunks": chunks,
        "segs": segs,
        "last_end": last_end,
        "nseg": nseg,
        "idx16": idx16,
        "lanef": lanef,
        "slot_all": slot_all,
    }


def build_nc(layout, ncores=N_CORES):
    t_total = layout["t_total"]
    td = t_total + (t_total & 1)  # even for int32 blob packing
    nc = bacc.Bacc("TRN2", target_bir_lowering=False, num_devices=ncores,
                   dynamic_dma_scratch_size=32768, num_swdge_queues=4)

    kf = F // 128  # contraction chunks for x @ W
    # Packed constant blob (int32 cols): dstf[td/2] | iota[64] | deg[PB]
    #                                    | bias[C] | w[kf*C/2]
    cb = td // 2 + 64 + PB + C + kf * C // 2
    xt_in = nc.dram_tensor("xt_sh", [F, NP], bf16, kind="ExternalInput")
    cb_in = nc.dram_tensor("cblob", [128, cb], i32, kind="ExternalInput")
    idx_in = nc.dram_tensor("idxs", [128, t_total * 8], i16, kind="ExternalInput")
    out_t = nc.dram_tensor("out", [NP, C], f32, kind="ExternalOutput")

    with tile.TileContext(nc) as tc, ExitStack() as ctx:
        const = ctx.enter_context(tc.tile_pool(name="const", bufs=1))
        dram = ctx.enter_context(tc.tile_pool(name="dram", bufs=1, space="DRAM"))

        blob = const.tile([128, cb], i32)
        nc.sync.dma_start(out=blob[:], in_=cb_in[:, :])
        o1 = td // 2
        o2 = o1 + 64
        o3 = o2 + PB
        o4 = o3 + C
        dstf = blob[:, 0:o1].bitcast(bf16)           # [128, td]
        iota = blob[:, o1:o2].bitcast(bf16)          # [128, 128]
        deg_t = blob[:, o2:o3].bitcast(f32)          # [128, PB]
        bias_t = blob[:, o3:o4].bitcast(f32)         # [128, C]
        w_bf = blob[:, o4:cb].bitcast(bf16)          # [128, kf*C]

        diss = const.tile([128, PB], f32)   # deg^-1/2
        d2 = const.tile([128, PB], f32)     # deg^-1
        alph = const.tile([128, PB], f32)   # deg^-3/2
        nc.vector.reciprocal(d2[:], deg_t)
        nc.scalar.activation(diss[:], d2[:], AF.Sqrt)
        nc.vector.tensor_mul(alph[:], d2[:], diss[:])

        yself = const.tile([128, PB * C], f32)  # xw * deg^-1/2 (self-loop)
        acc = const.tile([128, PB * C], f32)    # aggregated messages

        y_dup = dram.tile([NP, 2 * C], bf16)
        y_ga = dram.tile([4 * NP, 2 * C], bf16, addr_space="Shared")
        y_gb = dram.tile([4 * NP, 2 * C], bf16, addr_space="Shared")

        # ---- Phase A: xw = x @ W, y = xw * diss -> strided y table ----
        tw = 7
        xt3 = xt_in.ap().rearrange("(k p) n -> p k n", p=128)
        with (
            tc.tile_pool(name="xa", bufs=2) as xa,
            tc.tile_pool(name="psA", bufs=4, space="PSUM") as psa,
            tc.tile_pool(name="ya", bufs=2) as yap,
        ):
            for tg in range(PB // tw):
                xg = xa.tile([128, kf, tw * 128], bf16)
                nc.sync.dma_start(
                    out=xg[:],
                    in_=xt3[:, :, tg * tw * 128:(tg + 1) * tw * 128],
                )
                ybg = yap.tile([128, tw * C], bf16)
                for j in range(tw):
                    t = tg * tw + j
                    ps_xw = psa.tile([128, C], f32, tag="psxw")
                    for k in range(kf):
                        nc.tensor.matmul(
                            ps_xw[:],
                            lhsT=xg[:, k, j * 128:(j + 1) * 128],
                            rhs=w_bf[:, k * C:(k + 1) * C],
                            start=(k == 0), stop=(k == kf - 1),
                        )
                    nc.vector.tensor_scalar_mul(
                        ybg[:, j * C:(j + 1) * C], ps_xw[:], diss[:, t:t + 1]
                    )
                    nc.vector.tensor_scalar_mul(
                        yself[:, t * C:(t + 1) * C], ps_xw[:], diss[:, t:t + 1]
                    )
                nc.sync.dma_start(
                    out=y_dup[tg * tw * 128:(tg + 1) * tw * 128, 0:C].rearrange(
                        "(g p) c -> p g c", p=128
                    ),
                    in_=ybg[:].rearrange("p (g c) -> p g c", c=C),
                )

        # ---- Phase B: replicate y table (split so B overlaps Phase C) ----
        nc.gpsimd.collective_compute(
            "AllGather",
            mybir.AluOpType.bypass,
            replica_groups=[list(range(ncores))],
            ins=[y_dup[0:HALF, :].opt()],
            outs=[y_ga[:].opt()],
        )
        nc.gpsimd.collective_compute(
            "AllGather",
            mybir.AluOpType.bypass,
            replica_groups=[list(range(ncores))],
            ins=[y_dup[HALF:NP, :].opt()],
            outs=[y_gb[:].opt()],
        )
        gsrc = [y_ga[0:GROUP, :], y_ga[GROUP:4 * NP, :],
                y_gb[0:GROUP, :], y_gb[GROUP:4 * NP, :]]

        # ---- Phase C: gather + aggregate + epilogue ----
        segs = layout["segs"]
        last_end = layout["last_end"]
        nseg = layout["nseg"]
        seg_i = 0
        ps_open = {}
        done_blocks = []
        og = None
        og_blocks = []
        call_no = [0]

        with (
            tc.tile_pool(name="idxp", bufs=3) as idxp,
            tc.tile_pool(name="gp", bufs=3) as gp,
            tc.tile_pool(name="ohp", bufs=3) as ohp,
            tc.tile_pool(name="psC", bufs=8, space="PSUM") as psc,
            tc.tile_pool(name="ep", bufs=3) as ep,
            tc.tile_pool(name="ogp", bufs=2) as ogp,
        ):
            def epilogue(b):
                nonlocal og, og_blocks
                v = ep.tile([128, C], f32, tag="v")
                if nseg[b] > 0:
                    nc.vector.tensor_add(
                        v[:], acc[:, b * C:(b + 1) * C],
                        yself[:, b * C:(b + 1) * C],
                    )
                else:
                    nc.scalar.copy(v[:], yself[:, b * C:(b + 1) * C])
                nc.vector.tensor_scalar(
                    v[:], v[:], alph[:, b:b + 1], None,
                    op0=mybir.AluOpType.mult,
                )
                nc.vector.tensor_add(v[:], v[:], bias_t)
                nm = ep.tile([128, 1], f32, tag="nm")
                nc.vector.reduce_max(
                    nm[:], v[:], axis=mybir.AxisListType.X, negate=True
                )
                ex = ep.tile([128, C], f32, tag="ex")
                z = ep.tile([128, 1], f32, tag="z")
                nc.scalar.activation(
                    ex[:], v[:], AF.Exp, bias=nm[:], scale=1.0, accum_out=z[:]
                )
                lz = ep.tile([128, 1], f32, tag="lz")
                nc.scalar.activation(lz[:], z[:], AF.Ln)
                c0 = ep.tile([128, 1], f32, tag="c0")
                nc.vector.tensor_sub(c0[:], nm[:], lz[:])
                if og is None:
                    og = ogp.tile([128, EG * C], f32)
                    og_blocks = []
                oslot = len(og_blocks)
                nc.vector.tensor_scalar_add(
                    og[:, oslot * C:(oslot + 1) * C], v[:], c0[:]
                )
                og_blocks.append(b)
                if len(og_blocks) == EG or b == PB - 1:
                    b0 = og_blocks[0]
                    nb = len(og_blocks)
                    assert og_blocks == list(range(b0, b0 + nb))
                    nc.sync.dma_start(
                        out=out_t[b0 * 128:(b0 + nb) * 128, :].rearrange(
                            "(g p) c -> p g c", p=128
                        ),
                        in_=og[:, 0:nb * C].rearrange("p (g c) -> p g c", c=C),
                    )
                    og = None

            for ci, (t0, t1, calls) in enumerate(layout["chunks"]):
                tcn = t1 - t0
                idxt = idxp.tile([128, tcn * 8], i16)
                nc.sync.dma_start(out=idxt[:], in_=idx_in[:, t0 * 8:t1 * 8])
                gbuf = gp.tile([128, tcn * 128], bf16)
                g3 = gbuf[:].rearrange("p (t e) -> p t e", e=128)
                for (g, ts_, te_, reg) in calls:
                    nc.gpsimd.dma_gather(
                        g3[:, ts_ - t0:te_ - t0, :],
                        gsrc[g],
                        idxt[:, (ts_ - t0) * 8:(te_ - t0) * 8],
                        (te_ - ts_) * 128, reg, 128,
                        queue_num=call_no[0] % 4,
                    )
                    call_no[0] += 1
                oh = ohp.tile([128, tcn * 128], bf16)
                oh3 = oh[:].rearrange("p (t l) -> p t l", l=128)
                d3 = (
                    dstf[:, t0:t1]
                    .rearrange("p (t o) -> p t o", o=1)
                    .to_broadcast([128, tcn, 128])
                )
                i3 = (
                    iota[:]
                    .rearrange("p (o l) -> p o l", o=1)
                    .to_broadcast([128, tcn, 128])
                )
                nc.vector.tensor_tensor(
                    out=oh3, in0=d3, in1=i3, op=mybir.AluOpType.is_equal
                )
                # matmuls for all segment pieces inside this chunk
                while seg_i < len(segs) and segs[seg_i][1] < t1:
                    b, s0, s1 = segs[seg_i]
                    if s0 >= t0 and seg_i not in ps_open:
                        ps_open[seg_i] = psc.tile(
                            [128, C], f32, tag="agg", name=f"agg{seg_i}"
                        )
                    pss = ps_open[seg_i]
                    for t in range(max(s0, t0), min(s1, t1)):
                        nc.tensor.matmul(
                            pss[:],
                            lhsT=oh3[:, t - t0, :],
                            rhs=g3[:, t - t0, 0:C],
                            start=(t == s0),
                            stop=(t == s1 - 1),
                        )
                    if s1 > t1:
                        break  # segment continues in next chunk
                    # drain psum into acc
                    first = not done_blocks or all(
                        bb != b for bb in done_blocks
                    )
                    if first:
                        nc.scalar.copy(acc[:, b * C:(b + 1) * C], pss[:])
                    else:
                        nc.vector.tensor_add(
                            acc[:, b * C:(b + 1) * C], pss[:],
                            acc[:, b * C:(b + 1) * C],
                        )
                    done_blocks.append(b)
                    del ps_open[seg_i]
                    if last_end[b] == s1:
                        epilogue(b)
                    seg_i += 1
            for b in range(PB):
                if nseg[b] == 0:
                    epilogue(b)

    nc.compile()
    return nc


def host_prep(x, edge_index, W, b, layout):
    """Pure index/layout preprocessing. Returns per-core input maps."""
    src = np.asarray(edge_index[0], dtype=np.int64)
    dst = np.asarray(edge_index[1], dtype=np.int64)
    deg = (np.bincount(dst, minlength=N) + 1).astype(np.float32)

    t_total = layout["t_total"]
    td = t_total + (t_total & 1)
    kf = F // 128

    iota_arr = np.broadcast_to(
        np.arange(128, dtype=np.float32), (128, 128)
    ).astype(ml_dtypes.bfloat16).copy()
    bias_rep = np.broadcast_to(
        np.asarray(b, dtype=np.float32), (128, C)
    ).astype(np.float32).copy()
    w_arr = np.ascontiguousarray(
        np.asarray(W, dtype=np.float32)
        .reshape(kf, 128, C)
        .transpose(1, 0, 2)
        .astype(ml_dtypes.bfloat16)
    ).reshape(128, kf * C)
    x_bf = np.asarray(x, dtype=np.float32).astype(ml_dtypes.bfloat16)

    in_maps = []
    for c in range(N_CORES):
        sa = layout["slot_all"][c]
        xt_sh = np.zeros((F, NP), dtype=ml_dtypes.bfloat16)
        xt_sh[:, sa] = x_bf[c * NSH:(c + 1) * NSH].T
        deg_slot = np.ones(NP, dtype=np.float32)
        deg_slot[sa] = deg[c * NSH:(c + 1) * NSH]
        deg_sh = np.ascontiguousarray(deg_slot.reshape(PB, 128).T)

        dstf = np.zeros((128, td), dtype=ml_dtypes.bfloat16)
        lf = layout["lanef"][c].reshape(t_total, 128).T  # [128, t_total]
        dstf[:, :t_total] = lf.astype(ml_dtypes.bfloat16)

        blob = np.concatenate(
            [
                dstf.view(np.uint8),
                iota_arr.view(np.uint8),
                deg_sh.view(np.uint8),
                bias_rep.view(np.uint8),
                w_arr.view(np.uint8),
            ],
            axis=1,
        ).view(np.int32)

        idx = layout["idx16"][c]  # [t_total*128]
        idx_tile = np.tile(
            idx.reshape(-1, 16).T, (8, 1)
        ).astype(np.int16)  # [128, t_total*8]

        in_maps.append({"xt_sh": xt_sh, "cblob": blob, "idxs": idx_tile})
    return in_maps


def run(x, edge_index, W, b, trace=False, **spmd_kwargs):
    layout = build_layout(edge_index)
    in_maps = host_prep(x, edge_index, W, b, layout)
    nc = build_nc(layout)
    res = bass_utils.run_bass_kernel_spmd(
        nc, in_maps, core_ids=list(range(N_CORES)), trace=trace, **spmd_kwargs
    )
    out = np.concatenate(
        [res.results[c]["out"][layout["slot_all"][c]] for c in range(N_CORES)],
        axis=0,
    )
    return out, res


def kernel(x, edge_index, W, b):
    out, _ = run(x, edge_index, W, b)
    return out
